# revision 4
# baseline (speedup 1.0000x reference)
"""ArcFace margin loss (ArcMarginLoss) on 8 Trainium2 NeuronCores.

Classification-parallel sharding: the class dimension V=32000 of W is split
across the 8 cores (4000 classes each; tile padding to 4096 exists only for
the transposes - padded classes are excluded from the matmul/exp domain).

Per core (one SPMD NEFF, per-core data via inputs):
  - W pipeline (32 row tiles): sum-of-squares (DVE) -> Newton rsqrt ->
    prescale by 8/|W_row| with an fp8 cast and a pair-interleaving scatter
    (one DVE op), then PE-transpose the fp8 bytes as fp16 *pairs* (a
    bit-exact byte mover, 2 transposes per tile) and copy out as u16.  The
    result nwP[k, h, v] packs the contraction pair (d=256h+k, d=256h+128+k)
    of class v in adjacent bytes - exactly the DoubleRow moving-operand
    pair layout.
  - x pipeline (16 row tiles): sum-of-squares, PE-transpose the raw bf16
    rows, cast to fp8 in the PSUM->SBUF copy -> nxT [P, KT, B] (the
    chunk-strided pair layout LDWEIGHTS requires).  The x norm s/(8|x|)
    is folded into the exp scale, so x is matmul'd raw.
  - Cosine block [2048 x 4000] via fp8 DoubleRow matmuls (256 contraction
    rows per instruction, 2x the bf16 PE rate), fp32 PSUM accumulate.
  - exp(s*cos - 30) + row-sum fused in one scalar-engine activation per
    (m-tile, chunk), computed IN PLACE on PSUM (widths 928/1536/1536 per
    row, 48 activations total).  cos <= 1 so the shifted logits never
    overflow: no max pass, no cross-core collective.
  - Label columns: the host gathers x[i] / W[label_i] for the <=512 rows
    this core owns; the core computes cos_y fp32, phi = cos(theta+m), and
    emits delta = exp(s*phi-30) - exp(s*cos_y-30) and t = s*phi.

Host epilogue: S = sum_c S_c (+ scatter-add of deltas),
loss = mean(30 + log(S) - s*phi_label).
"""

import math
import numpy as np
from contextlib import ExitStack

import concourse.bass as bass
import concourse.tile as tile
from concourse import bacc, mybir
from concourse import bass_utils
from concourse._compat import with_exitstack
from concourse.masks import make_identity

P = 128
B = 2048          # batch rows
D = 512           # feature dim
V = 32000         # classes
NCORES = 8
VS = V // NCORES  # 4000 classes per core
VSP = 4096        # padded shard size (32 tiles of 128)
MT = B // P       # 16 row tiles
KT = D // P       # 4 contraction tiles
WTILES = VSP // P  # 32 W tiles per core
NT = 3            # exp chunks per row
CH_OFF = [0, 928, 2464]       # chunk start columns
CH_W = [928, 1536, 1536]      # chunk widths (sum = VS = 4000)
GCAP = 512        # capacity for host-gathered label rows per core

S_SCALE = 30.0
M_MARGIN = 0.5
SHIFT = 30.0      # exp(logit - SHIFT): logits <= 30 so always <= 0
EPS = 1e-12
WSC = 8.0         # power-of-2 prescale on normalized W (fp8 headroom)

# Schraudolph exp constants (optional DVE offload of exp chunks).
LOG2E = 1.4426950408889634
SCH_A = float(1 << 23) * LOG2E
SCH_B = 1064870319.2

F32 = mybir.dt.float32
BF16 = mybir.dt.bfloat16
FP8 = mybir.dt.float8e4
F16 = mybir.dt.float16
I32 = mybir.dt.int32
AX = mybir.AxisListType
OP = mybir.AluOpType
AF = mybir.ActivationFunctionType
PM = mybir.MatmulPerfMode

# (g, m) chunks whose exp runs on DVE (Schraudolph) instead of ACT.
DVE_CHUNKS = set()


@with_exitstack
def _arc_kernel(ctx: ExitStack, tc: tile.TileContext,
                x_d: bass.AP, w_d: bass.AP, xg_d: bass.AP, wg_d: bass.AP,
                own_d: bass.AP, s_d: bass.AP, d_d: bass.AP, t_d: bass.AP):
    nc = tc.nc
    cos_m = math.cos(M_MARGIN)
    sin_m = math.sin(M_MARGIN)

    sb = ctx.enter_context(tc.tile_pool(name="sb", bufs=1))
    ld = ctx.enter_context(tc.tile_pool(name="ld", bufs=16))
    wld = ctx.enter_context(tc.tile_pool(name="wld", bufs=14))
    w8p = ctx.enter_context(tc.tile_pool(name="w8p", bufs=4))
    gld = ctx.enter_context(tc.tile_pool(name="gld", bufs=1))
    scr = ctx.enter_context(tc.tile_pool(name="scr", bufs=4))
    nsc = ctx.enter_context(tc.tile_pool(name="nsc", bufs=3))
    sch = ctx.enter_context(tc.tile_pool(name="sch", bufs=2))
    ps = ctx.enter_context(tc.tile_pool(name="ps", bufs=2, space="PSUM"))
    pst = ctx.enter_context(tc.tile_pool(name="pst", bufs=2, space="PSUM"))

    GT = GCAP // P      # 4 tiles of gathered label rows
    NCHUNK = max(CH_W)

    # persistent SBUF tensors
    nxT = sb.tile([P, KT, B], FP8)       # x^T (K-major, fp8)
    nwP = sb.tile([P, 2, VSP], F16)      # packed-pair (8/|W|)W^T (fp8 x2)
    ident = sb.tile([P, P], BF16)
    make_identity(nc, ident)
    ident16 = sb.tile([P, P], F16)
    nc.vector.tensor_copy(out=ident16, in_=ident)

    nbias = sb.tile([P, 1], F32)         # -SHIFT bias for all the exp ops
    nc.vector.memset(nbias, -SHIFT)
    magic = sb.tile([P, 1], I32)         # quake rsqrt seed constant
    nc.vector.memset(magic, 0x5F3759DF)

    rx = sb.tile([P, MT], F32)           # s/(8|x_row|) per batch row
    rxA = sb.tile([P, MT], F32)          # rx * SCH_A (DVE exp scale)
    rw = sb.tile([P, WTILES], F32)
    Spart = sb.tile([P, MT, NT], F32)    # per-chunk exp row sums
    Sacc = sb.tile([P, MT], F32)         # partial sums per row (p-major)
    own_t = sb.tile([P, GT], F32)        # validity mask for gathered rows

    nc.sync.dma_start(out=own_t, in_=own_d.rearrange("(p m) -> p m", p=P))

    def sumsq(src_tile, ssq_col):
        """row sum-of-squares in one DVE op (scratch out is discarded)."""
        sq = scr.tile([P, D], src_tile.dtype, tag="sq", name="sq")
        nc.vector.scalar_tensor_tensor(
            out=sq, in0=src_tile, scalar=1.0, in1=src_tile,
            op0=OP.mult, op1=OP.mult, accum_out=ssq_col)

    def rsqrt_newton(vec, gb, iters=2):
        """in-place 1/sqrt(vec) on DVE only (no ACT table traffic)."""
        yi = nsc.tile([P, max(MT, 2 * GT)], I32, tag="nt_y", name="nt_y")[:, :gb]
        nc.vector.tensor_scalar(yi, vec.bitcast(I32), 1, None,
                                OP.arith_shift_right)
        nc.vector.tensor_tensor(yi, magic.to_broadcast([P, gb]), yi,
                                OP.subtract)
        y = yi.bitcast(F32)
        xh = nsc.tile([P, max(MT, 2 * GT)], F32, tag="nt_xh", name="nt_xh")[:, :gb]
        nc.vector.tensor_scalar_mul(xh, vec, 0.5)
        p = nsc.tile([P, max(MT, 2 * GT)], F32, tag="nt_p", name="nt_p")[:, :gb]
        for it in range(iters):
            nc.vector.tensor_tensor(p, y, y, OP.mult)
            nc.vector.tensor_tensor(p, p, xh, OP.mult)
            nc.vector.tensor_scalar(p, p, -1.0, 1.5, OP.mult, OP.add)
            nc.vector.tensor_tensor(y if it < iters - 1 else vec, y, p, OP.mult)

    def x_chain(i, xt):
        """sumsq + transpose + fp8-cast-copy for one x tile."""
        sumsq(xt, rx[:, i:i + 1])
        pt = pst.tile([P, KT, P], BF16, tag="tpsum", name="xtp")
        for k in range(KT):
            nc.tensor.transpose(pt[:, k], xt[:, k * P:(k + 1) * P], ident)
        nc.vector.tensor_copy(out=nxT[:, :, i * P:(i + 1) * P], in_=pt)

    def w_fin(t, wt):
        """prescale W tile into fp8 with pair-interleave scatter, then
        2 packed transposes + one u16 copy into nwP."""
        wp8 = w8p.tile([P, D], FP8, tag="wp8", name="wp8")
        # out byte pos = 256*b + 2*k + i for input d = 256*b + 128*i + k
        nc.vector.tensor_scalar_mul(
            wp8.rearrange("p (b k i) -> p b i k", b=2, k=P, i=2),
            wt.rearrange("p (b i k) -> p b i k", b=2, i=2, k=P),
            rw[:, t:t + 1])
        w16 = wp8.bitcast(F16)           # [P, 256] u16 pair view
        pt = pst.tile([P, 2, P], F16, tag="tpsum", name="wtp")
        for b in range(2):
            nc.tensor.transpose(pt[:, b], w16[:, b * P:(b + 1) * P], ident16)
        nc.vector.tensor_copy(out=nwP[:, :, t * P:(t + 1) * P], in_=pt)

    # PE warm-up: dependency-free transposes keep the HAM activity window
    # busy so the PE clock-gate is at 8/8 when the first real matmuls arrive.
    for _ in range(22):
        wp = pst.tile([P, KT, P], BF16, tag="tpsum", name="warm")
        nc.tensor.transpose(wp[:, 0], ident, ident)

    # ---- prefix: x tiles 0-7 and W tiles 0-7 (chunk 0 needs 8 W tiles) ----
    xrows0 = [None] * 8
    wrows0 = [None] * 8
    order = ([("x", 0)] + [("w", i) for i in range(4)]
             + [("x", i) for i in range(1, 4)]
             + [("w", i) for i in range(4, 8)]
             + [("x", i) for i in range(4, 8)])
    for kind, i in order:
        if kind == "x":
            xt = ld.tile([P, D], BF16, tag="xload", name="xload")
            nc.sync.dma_start(out=xt, in_=x_d[i * P:(i + 1) * P, :])
            xrows0[i] = xt
        else:
            wt = wld.tile([P, D], BF16, tag="wload", name="wload")
            nc.sync.dma_start(out=wt, in_=w_d[i * P:(i + 1) * P, :])
            wrows0[i] = wt
    for i in range(8):
        sumsq(wrows0[i], rw[:, i:i + 1])
    nc.vector.tensor_scalar(rw[:, 0:8], rw[:, 0:8],
                            1.0 / (WSC * WSC), EPS * EPS, OP.mult, OP.max)
    rsqrt_newton(rw[:, 0:8], 8)
    for i in range(8):
        w_fin(i, wrows0[i])
        x_chain(i, xrows0[i])
    nc.vector.tensor_scalar(rx[:, 0:8], rx[:, 0:8],
                            (WSC * WSC) / (S_SCALE * S_SCALE), EPS * EPS,
                            OP.mult, OP.max)
    rsqrt_newton(rx[:, 0:8], 8)

    # ---- x group 1 (tiles 8-15) ----
    def x_group1():
        xrows = []
        for m in range(8, MT):
            xt = ld.tile([P, D], BF16, tag="xload", name="xload")
            nc.sync.dma_start(out=xt, in_=x_d[m * P:(m + 1) * P, :])
            xrows.append(xt)
        for i, m in enumerate(range(8, MT)):
            x_chain(m, xrows[i])
        nc.vector.tensor_scalar(rx[:, 8:MT], rx[:, 8:MT],
                                (WSC * WSC) / (S_SCALE * S_SCALE),
                                EPS * EPS, OP.mult, OP.max)
        rsqrt_newton(rx[:, 8:MT], 8)
        if DVE_CHUNKS:
            nc.vector.tensor_scalar_mul(rxA, rx, SCH_A)

    def w_group(t0, t1):
        rows = []
        for t in range(t0, t1):
            wt = wld.tile([P, D], BF16, tag="wload", name="wload")
            nc.sync.dma_start(out=wt, in_=w_d[t * P:(t + 1) * P, :])
            rows.append(wt)
        for i, t in enumerate(range(t0, t1)):
            sumsq(rows[i], rw[:, t:t + 1])
        nc.vector.tensor_scalar(rw[:, t0:t1], rw[:, t0:t1],
                                1.0 / (WSC * WSC), EPS * EPS, OP.mult, OP.max)
        rsqrt_newton(rw[:, t0:t1], t1 - t0)
        for i, t in enumerate(range(t0, t1)):
            w_fin(t, rows[i])

    nw8 = nwP.bitcast(FP8)               # [P, 2, 2*VSP] byte view

    def mm_chunk(g, last=False):
        v0, cw = CH_OFF[g], CH_W[g]
        for m in range(MT):
            pm = ps.tile([P, NCHUNK], F32, tag="mm")
            n0 = 0
            while n0 < cw:
                nw_ = min(512, cw - n0)
                for h in range(2):
                    rhs = nw8[:, h, 2 * (v0 + n0):2 * (v0 + n0 + nw_)]
                    nc.tensor.matmul(
                        pm[:, n0:n0 + nw_],
                        nxT[:, 2 * h:2 * h + 2, m * P:(m + 1) * P],
                        rhs.rearrange("p (n two) -> p two n", two=2),
                        start=(h == 0), stop=(h == 1),
                        perf_mode=PM.DoubleRow)
                n0 += nw_
            if (g, m) in DVE_CHUNKS:
                si = sch.tile([P, NCHUNK], I32, tag="schi", name="schi")[:, :cw]
                nc.vector.tensor_scalar(
                    si, pm[:, :cw], rxA[:, m:m + 1],
                    SCH_B - SHIFT * SCH_A, OP.mult, OP.add)
                nc.vector.tensor_reduce(
                    out=Spart[:, m, g:g + 1], in_=si.bitcast(F32),
                    axis=AX.X, op=OP.add)
            else:
                nc.scalar.activation(
                    out=pm[:, :cw], in_=pm[:, :cw], func=AF.Exp,
                    bias=nbias, scale=rx[:, m:m + 1],
                    accum_out=Spart[:, m, g:g + 1])
            if last:
                nc.vector.tensor_reduce(
                    out=Sacc[:, m:m + 1], in_=Spart[:, m, :],
                    axis=AX.X, op=OP.add)

    # ---- compact label chain: xg/wg are host-gathered label rows ----
    cosy = sb.tile([P, GT], F32)
    delta = sb.tile([P, GT], F32)
    tvec = sb.tile([P, GT], F32)

    def wg_chain():
        rgg = sb.tile([P, 2 * GT], F32)
        dots = sb.tile([P, GT], F32)
        xq = gld.tile([P, GT, D], F32, tag="xgload", name="xgload")
        nc.sync.dma_start(out=xq, in_=xg_d.rearrange("(g p) d -> p g d", p=P))
        wq = gld.tile([P, GT, D], F32, tag="wgload", name="wgload")
        nc.sync.dma_start(out=wq, in_=wg_d.rearrange("(g p) d -> p g d", p=P))
        pairs = []
        for i in range(GT):
            xt, wt = xq[:, i, :], wq[:, i, :]
            sumsq(xt, rgg[:, i:i + 1])
            sumsq(wt, rgg[:, GT + i:GT + i + 1])
            pairs.append((xt, wt))
        for i, (xt, wt) in enumerate(pairs):
            sq2 = scr.tile([P, D], F32, tag="sq2")
            nc.vector.scalar_tensor_tensor(
                out=sq2, in0=xt, scalar=1.0, in1=wt,
                op0=OP.mult, op1=OP.mult, accum_out=dots[:, i:i + 1])
        nc.vector.tensor_scalar_max(rgg, rgg, EPS * EPS)
        rsqrt_newton(rgg, 2 * GT)
        nc.vector.tensor_tensor(cosy, dots, rgg[:, 0:GT], OP.mult)
        nc.vector.tensor_tensor(cosy, cosy, rgg[:, GT:2 * GT], OP.mult)

    def phi_chain():
        # mphi = sin*sin_m - cosy*cos_m = -phi
        sq = sb.tile([P, GT], F32)
        nc.vector.tensor_tensor(sq, cosy, cosy, OP.mult)
        om = sb.tile([P, GT], F32)
        nc.vector.tensor_scalar(om, sq, -1.0, 1.0, OP.mult, OP.add)
        nc.vector.tensor_scalar_max(om, om, 0.0)
        rsin = sb.tile([P, GT], F32)
        nc.vector.tensor_scalar_max(rsin, om, 1e-30)
        rsqrt_newton(rsin, GT)
        sin = sb.tile([P, GT], F32)
        nc.vector.tensor_tensor(sin, om, rsin, OP.mult)
        cm = sb.tile([P, GT], F32)
        nc.vector.tensor_scalar_mul(cm, cosy, cos_m)
        mphi = sb.tile([P, GT], F32)
        nc.vector.scalar_tensor_tensor(
            out=mphi, in0=sin, scalar=sin_m, in1=cm,
            op0=OP.mult, op1=OP.subtract)

        expphi = sb.tile([P, GT], F32)
        nc.scalar.activation(out=expphi, in_=mphi, func=AF.Exp,
                             bias=nbias, scale=-S_SCALE)
        expcos = sb.tile([P, GT], F32)
        nc.scalar.activation(out=expcos, in_=cosy, func=AF.Exp,
                             bias=nbias, scale=S_SCALE)
        nc.vector.tensor_tensor(delta, expphi, expcos, OP.subtract)
        nc.vector.tensor_tensor(delta, delta, own_t, OP.mult)
        nc.vector.tensor_scalar_mul(tvec, mphi, -S_SCALE)
        nc.vector.tensor_tensor(tvec, tvec, own_t, OP.mult)

    # ---- emission schedule: PE-dense, DVE feeds one W group ahead ----
    x_group1()
    w_group(8, 14)
    w_group(14, 20)
    mm_chunk(0)
    w_group(20, 26)
    w_group(26, 32)
    mm_chunk(1)
    wg_chain()
    phi_chain()
    mm_chunk(2, last=True)

    # ---- tail: write p-major outputs ----
    nc.sync.dma_start(out=s_d.rearrange("(p m) -> p m", p=P), in_=Sacc)
    nc.sync.dma_start(out=d_d.rearrange("(p m) -> p m", p=P), in_=delta)
    nc.sync.dma_start(out=t_d.rearrange("(p m) -> p m", p=P), in_=tvec)


def build_bass():
    nc = bacc.Bacc("TRN2", target_bir_lowering=False, debug=False,
                   enable_asserts=False, num_devices=NCORES)
    x_d = nc.dram_tensor("x_in", [B, D], BF16, kind="ExternalInput").ap()
    w_d = nc.dram_tensor("w_shard", [VSP, D], BF16, kind="ExternalInput").ap()
    xg_d = nc.dram_tensor("x_gather", [GCAP, D], F32, kind="ExternalInput").ap()
    wg_d = nc.dram_tensor("w_gather", [GCAP, D], F32, kind="ExternalInput").ap()
    own_d = nc.dram_tensor("owned", [GCAP], F32, kind="ExternalInput").ap()
    s_d = nc.dram_tensor("s_out", [B], F32, kind="ExternalOutput").ap()
    d_d = nc.dram_tensor("d_out", [GCAP], F32, kind="ExternalOutput").ap()
    t_d = nc.dram_tensor("t_out", [GCAP], F32, kind="ExternalOutput").ap()
    with tile.TileContext(nc) as tc:
        _arc_kernel(tc, x_d, w_d, xg_d, wg_d, own_d, s_d, d_d, t_d)
    nc.compile()
    return nc


_NC = None


def _get_nc():
    global _NC
    if _NC is None:
        _NC = build_bass()
    return _NC


def _pm(vec, nt):
    """host-side inverse of the device's p-major [(p, m)] output layout."""
    return vec.reshape(P, nt).T.reshape(-1)


def make_in_maps(x: np.ndarray, W: np.ndarray, labels: np.ndarray):
    import ml_dtypes
    x = np.ascontiguousarray(x, dtype=np.float32)
    W = np.ascontiguousarray(W, dtype=np.float32)
    x16 = x.astype(ml_dtypes.bfloat16)
    W16 = W.astype(ml_dtypes.bfloat16)
    lab = np.asarray(labels).astype(np.int64)
    shard_of = lab // VS
    in_maps = []
    idxs = []
    for c in range(NCORES):
        wsh = np.zeros((VSP, D), dtype=ml_dtypes.bfloat16)
        wsh[:VS] = W16[c * VS:(c + 1) * VS]
        idx = np.nonzero(shard_of == c)[0]
        assert len(idx) <= GCAP, f"core {c} owns {len(idx)} rows > {GCAP}"
        idxs.append(idx)
        xg = np.zeros((GCAP, D), dtype=np.float32)
        wg = np.zeros((GCAP, D), dtype=np.float32)
        xg[:len(idx)] = x[idx]
        wg[:len(idx)] = W[lab[idx]]
        owned = np.zeros(GCAP, dtype=np.float32)
        owned[:len(idx)] = 1.0
        # device reads owned as [(p, m)] p-major
        owned_pm = owned.reshape(GCAP // P, P).T.reshape(-1).copy()
        in_maps.append({
            "x_in": x16,
            "w_shard": wsh,
            "x_gather": xg,
            "w_gather": wg,
            "owned": owned_pm,
        })
    return in_maps, idxs


def combine_outputs(results, idxs):
    S = np.zeros(B, dtype=np.float64)
    t = np.zeros(B, dtype=np.float64)
    for c, r in enumerate(results):
        S += _pm(r["s_out"], MT).astype(np.float64)
    for c, r in enumerate(results):
        idx = idxs[c]
        S[idx] += _pm(r["d_out"], GCAP // P).astype(np.float64)[:len(idx)]
        t[idx] = _pm(r["t_out"], GCAP // P).astype(np.float64)[:len(idx)]
    loss = np.mean(SHIFT + np.log(S) - t)
    return np.asarray(loss, dtype=np.float32)


def kernel(x, W, labels, **run_kwargs):
    x = np.asarray(x)
    W = np.asarray(W)
    labels = np.asarray(labels)
    assert x.shape == (B, D) and W.shape == (V, D) and labels.shape == (B,), \
        (x.shape, W.shape, labels.shape)
    nc = _get_nc()
    in_maps, idxs = make_in_maps(x, W, labels)
    res = bass_utils.run_bass_kernel_spmd(
        nc, in_maps, core_ids=list(range(NCORES)), **run_kwargs)
    out = combine_outputs(res.results, idxs)
    kernel.last_results = res
    return out


# revision 5
# speedup vs baseline: 1.1349x; 1.1349x over previous
"""ArcFace margin loss (ArcMarginLoss) on 8 Trainium2 NeuronCores.

Classification-parallel sharding: the class dimension V=32000 of W is split
across the 8 cores (4000 classes each; tile padding to 4096 exists only for
the transposes - padded classes are excluded from the matmul/exp domain).

Per core (one SPMD NEFF, per-core data via inputs):
  - W pipeline (32 row tiles): sum-of-squares (DVE) -> Newton rsqrt ->
    prescale by 8/|W_row| with an fp8 cast and a pair-interleaving scatter
    (one DVE op), then PE-transpose the fp8 bytes as fp16 *pairs* (a
    bit-exact byte mover, 2 transposes per tile) and copy out as u16.  The
    result nwP[k, h, v] packs the contraction pair (d=256h+k, d=256h+128+k)
    of class v in adjacent bytes - exactly the DoubleRow moving-operand
    pair layout.
  - x pipeline (16 row tiles): sum-of-squares, PE-transpose the raw bf16
    rows, cast to fp8 in the PSUM->SBUF copy -> nxT [P, KT, B] (the
    chunk-strided pair layout LDWEIGHTS requires).  The x norm s/(8|x|)
    is folded into the exp scale, so x is matmul'd raw.
  - Cosine block [2048 x 4000] via fp8 DoubleRow matmuls (256 contraction
    rows per instruction, 2x the bf16 PE rate), fp32 PSUM accumulate.
  - exp(s*cos - 30) + row-sum fused in one scalar-engine activation per
    (m-tile, chunk), computed IN PLACE on PSUM (widths 928/1536/1536 per
    row, 48 activations total).  cos <= 1 so the shifted logits never
    overflow: no max pass, no cross-core collective.
  - Label columns: the host gathers x[i] / W[label_i] for the <=512 rows
    this core owns; the core computes cos_y fp32, phi = cos(theta+m), and
    emits delta = exp(s*phi-30) - exp(s*cos_y-30) and t = s*phi.

Host epilogue: S = sum_c S_c (+ scatter-add of deltas),
loss = mean(30 + log(S) - s*phi_label).
"""

import math
import numpy as np
from contextlib import ExitStack

import concourse.bass as bass
import concourse.tile as tile
from concourse import bacc, mybir
from concourse import bass_utils
from concourse._compat import with_exitstack
from concourse.masks import make_identity

P = 128
B = 2048          # batch rows
D = 512           # feature dim
V = 32000         # classes
NCORES = 8
VS = V // NCORES  # 4000 classes per core
VSP = 4096        # padded shard size (32 tiles of 128)
MT = B // P       # 16 row tiles
KT = D // P       # 4 contraction tiles
WTILES = VSP // P  # 32 W tiles per core
NT = 3            # exp chunks per row
CH_OFF = [0, 928, 2464]       # chunk start columns
CH_W = [928, 1536, 1536]      # chunk widths (sum = VS = 4000)
GCAP = 512        # capacity for host-gathered label rows per core

S_SCALE = 30.0
M_MARGIN = 0.5
SHIFT = 30.0      # exp(logit - SHIFT): logits <= 30 so always <= 0
EPS = 1e-12
WSC = 8.0         # power-of-2 prescale on normalized W (fp8 headroom)

# Schraudolph exp constants (optional DVE offload of exp chunks).
LOG2E = 1.4426950408889634
SCH_A = float(1 << 23) * LOG2E
SCH_B = 1064870319.2

F32 = mybir.dt.float32
BF16 = mybir.dt.bfloat16
FP8 = mybir.dt.float8e4
F16 = mybir.dt.float16
I32 = mybir.dt.int32
AX = mybir.AxisListType
OP = mybir.AluOpType
AF = mybir.ActivationFunctionType
PM = mybir.MatmulPerfMode

# (g, m) chunks whose exp runs on DVE (Schraudolph) instead of ACT.
DVE_CHUNKS = set()


@with_exitstack
def _arc_kernel(ctx: ExitStack, tc: tile.TileContext,
                x_d: bass.AP, w_d: bass.AP, xg_d: bass.AP, wg_d: bass.AP,
                own_d: bass.AP, s_d: bass.AP, d_d: bass.AP, t_d: bass.AP):
    nc = tc.nc
    cos_m = math.cos(M_MARGIN)
    sin_m = math.sin(M_MARGIN)

    sb = ctx.enter_context(tc.tile_pool(name="sb", bufs=1))
    ld = ctx.enter_context(tc.tile_pool(name="ld", bufs=16))
    wld = ctx.enter_context(tc.tile_pool(name="wld", bufs=14))
    w8p = ctx.enter_context(tc.tile_pool(name="w8p", bufs=4))
    gld = ctx.enter_context(tc.tile_pool(name="gld", bufs=1))
    scr = ctx.enter_context(tc.tile_pool(name="scr", bufs=4))
    nsc = ctx.enter_context(tc.tile_pool(name="nsc", bufs=3))
    sch = ctx.enter_context(tc.tile_pool(name="sch", bufs=2))
    exs = ctx.enter_context(tc.tile_pool(name="exs", bufs=3))
    ps = ctx.enter_context(tc.tile_pool(name="ps", bufs=2, space="PSUM"))
    pst = ctx.enter_context(tc.tile_pool(name="pst", bufs=2, space="PSUM"))

    GT = GCAP // P      # 4 tiles of gathered label rows
    NCHUNK = max(CH_W)

    # persistent SBUF tensors
    nxT = sb.tile([P, KT, B], FP8)       # x^T (K-major, fp8)
    nwT = sb.tile([P, KT, VSP], FP8)     # (8/|W|)W^T (K-major, fp8)
    ident = sb.tile([P, P], BF16)
    make_identity(nc, ident)

    nbias = sb.tile([P, 1], F32)         # -SHIFT bias for all the exp ops
    nc.vector.memset(nbias, -SHIFT)
    magic = sb.tile([P, 1], I32)         # quake rsqrt seed constant
    nc.vector.memset(magic, 0x5F3759DF)

    rx = sb.tile([P, MT], F32)           # s/(8|x_row|) per batch row
    rxA = sb.tile([P, MT], F32)          # rx * SCH_A (DVE exp scale)
    rw = sb.tile([P, WTILES], F32)
    Spart = sb.tile([P, MT, NT], F32)    # per-chunk exp row sums
    Sacc = sb.tile([P, MT], F32)         # partial sums per row (p-major)
    own_t = sb.tile([P, GT], F32)        # validity mask for gathered rows

    nc.sync.dma_start(out=own_t, in_=own_d.rearrange("(p m) -> p m", p=P))

    def sumsq(src_tile, ssq_col):
        """row sum-of-squares in one DVE op (scratch out is discarded)."""
        sq = scr.tile([P, D], src_tile.dtype, tag="sq", name="sq")
        nc.vector.scalar_tensor_tensor(
            out=sq, in0=src_tile, scalar=1.0, in1=src_tile,
            op0=OP.mult, op1=OP.mult, accum_out=ssq_col)

    def rsqrt_newton(vec, gb, iters=2):
        """in-place 1/sqrt(vec) on DVE only (no ACT table traffic)."""
        yi = nsc.tile([P, max(MT, 2 * GT)], I32, tag="nt_y", name="nt_y")[:, :gb]
        nc.vector.tensor_scalar(yi, vec.bitcast(I32), 1, None,
                                OP.arith_shift_right)
        nc.vector.tensor_tensor(yi, magic.to_broadcast([P, gb]), yi,
                                OP.subtract)
        y = yi.bitcast(F32)
        xh = nsc.tile([P, max(MT, 2 * GT)], F32, tag="nt_xh", name="nt_xh")[:, :gb]
        nc.vector.tensor_scalar_mul(xh, vec, 0.5)
        p = nsc.tile([P, max(MT, 2 * GT)], F32, tag="nt_p", name="nt_p")[:, :gb]
        for it in range(iters):
            nc.vector.tensor_tensor(p, y, y, OP.mult)
            nc.vector.tensor_tensor(p, p, xh, OP.mult)
            nc.vector.tensor_scalar(p, p, -1.0, 1.5, OP.mult, OP.add)
            nc.vector.tensor_tensor(y if it < iters - 1 else vec, y, p, OP.mult)

    def x_chain(i, xt):
        """sumsq + transpose + fp8-cast-copy for one x tile."""
        sumsq(xt, rx[:, i:i + 1])
        pt = pst.tile([P, KT, P], BF16, tag="tpsum", name="xtp")
        for k in range(KT):
            nc.tensor.transpose(pt[:, k], xt[:, k * P:(k + 1) * P], ident)
        nc.vector.tensor_copy(out=nxT[:, :, i * P:(i + 1) * P], in_=pt)

    def w_fin(t, wt):
        """prescale W tile (bf16), transpose, cast to fp8 on copy-out."""
        nwr = w8p.tile([P, D], BF16, tag="nwr", name="nwr")
        nc.vector.tensor_scalar_mul(nwr, wt, rw[:, t:t + 1])
        pt = pst.tile([P, KT, P], BF16, tag="tpsum", name="wtp")
        for k in range(KT):
            nc.tensor.transpose(pt[:, k], nwr[:, k * P:(k + 1) * P], ident)
        nc.vector.tensor_copy(out=nwT[:, :, t * P:(t + 1) * P], in_=pt)

    # PE warm-up: dependency-free transposes keep the HAM activity window
    # busy so the PE clock-gate is at 8/8 when the first real matmuls arrive.
    for _ in range(22):
        wp = pst.tile([P, KT, P], BF16, tag="tpsum", name="warm")
        nc.tensor.transpose(wp[:, 0], ident, ident)

    # ---- prefix: x tiles 0-7 and W tiles 0-7 (chunk 0 needs 8 W tiles) ----
    xrows0 = [None] * 8
    wrows0 = [None] * 8
    order = ([("x", 0)] + [("w", i) for i in range(4)]
             + [("x", i) for i in range(1, 4)]
             + [("w", i) for i in range(4, 8)]
             + [("x", i) for i in range(4, 8)])
    for kind, i in order:
        if kind == "x":
            xt = ld.tile([P, D], BF16, tag="xload", name="xload")
            nc.sync.dma_start(out=xt, in_=x_d[i * P:(i + 1) * P, :])
            xrows0[i] = xt
        else:
            wt = wld.tile([P, D], BF16, tag="wload", name="wload")
            nc.sync.dma_start(out=wt, in_=w_d[i * P:(i + 1) * P, :])
            wrows0[i] = wt
    for i in range(8):
        sumsq(wrows0[i], rw[:, i:i + 1])
    nc.vector.tensor_scalar(rw[:, 0:8], rw[:, 0:8],
                            1.0 / (WSC * WSC), EPS * EPS, OP.mult, OP.max)
    rsqrt_newton(rw[:, 0:8], 8)
    for i in range(8):
        w_fin(i, wrows0[i])
        x_chain(i, xrows0[i])
    nc.vector.tensor_scalar(rx[:, 0:8], rx[:, 0:8],
                            (WSC * WSC) / (S_SCALE * S_SCALE), EPS * EPS,
                            OP.mult, OP.max)
    rsqrt_newton(rx[:, 0:8], 8)

    # ---- x group 1 (tiles 8-15) ----
    def x_group1():
        xrows = []
        for m in range(8, MT):
            xt = ld.tile([P, D], BF16, tag="xload", name="xload")
            nc.sync.dma_start(out=xt, in_=x_d[m * P:(m + 1) * P, :])
            xrows.append(xt)
        for i, m in enumerate(range(8, MT)):
            x_chain(m, xrows[i])
        nc.vector.tensor_scalar(rx[:, 8:MT], rx[:, 8:MT],
                                (WSC * WSC) / (S_SCALE * S_SCALE),
                                EPS * EPS, OP.mult, OP.max)
        rsqrt_newton(rx[:, 8:MT], 8)
        if DVE_CHUNKS:
            nc.vector.tensor_scalar_mul(rxA, rx, SCH_A)

    def w_group(t0, t1):
        rows = []
        for t in range(t0, t1):
            wt = wld.tile([P, D], BF16, tag="wload", name="wload")
            nc.sync.dma_start(out=wt, in_=w_d[t * P:(t + 1) * P, :])
            rows.append(wt)
        for i, t in enumerate(range(t0, t1)):
            sumsq(rows[i], rw[:, t:t + 1])
        nc.vector.tensor_scalar(rw[:, t0:t1], rw[:, t0:t1],
                                1.0 / (WSC * WSC), EPS * EPS, OP.mult, OP.max)
        rsqrt_newton(rw[:, t0:t1], t1 - t0)
        for i, t in enumerate(range(t0, t1)):
            w_fin(t, rows[i])

    def mm_chunk(g, last=False):
        v0, cw = CH_OFF[g], CH_W[g]
        for m in range(MT):
            pm = ps.tile([P, NCHUNK], F32, tag="mm")
            n0 = 0
            while n0 < cw:
                nw_ = min(512, cw - n0)
                for h in range(2):
                    nc.tensor.matmul(
                        pm[:, n0:n0 + nw_],
                        nxT[:, 2 * h:2 * h + 2, m * P:(m + 1) * P],
                        nwT[:, 2 * h:2 * h + 2, v0 + n0:v0 + n0 + nw_],
                        start=(h == 0), stop=(h == 1),
                        perf_mode=PM.DoubleRow)
                n0 += nw_
            if (g, m) in DVE_CHUNKS:
                si = sch.tile([P, NCHUNK], I32, tag="schi", name="schi")[:, :cw]
                nc.vector.tensor_scalar(
                    si, pm[:, :cw], rxA[:, m:m + 1],
                    SCH_B - SHIFT * SCH_A, OP.mult, OP.add)
                nc.vector.tensor_reduce(
                    out=Spart[:, m, g:g + 1], in_=si.bitcast(F32),
                    axis=AX.X, op=OP.add)
            else:
                ex = exs.tile([P, NCHUNK], BF16, tag="ex", name="ex")[:, :cw]
                nc.scalar.activation(
                    out=ex, in_=pm[:, :cw], func=AF.Exp,
                    bias=nbias, scale=rx[:, m:m + 1],
                    accum_out=Spart[:, m, g:g + 1])
            if last:
                nc.vector.tensor_reduce(
                    out=Sacc[:, m:m + 1], in_=Spart[:, m, :],
                    axis=AX.X, op=OP.add)

    # ---- compact label chain: xg/wg are host-gathered label rows ----
    cosy = sb.tile([P, GT], F32)
    delta = sb.tile([P, GT], F32)
    tvec = sb.tile([P, GT], F32)

    def wg_chain():
        rgg = sb.tile([P, 2 * GT], F32)
        dots = sb.tile([P, GT], F32)
        xq = gld.tile([P, GT, D], F32, tag="xgload", name="xgload")
        nc.sync.dma_start(out=xq, in_=xg_d.rearrange("(g p) d -> p g d", p=P))
        wq = gld.tile([P, GT, D], F32, tag="wgload", name="wgload")
        nc.sync.dma_start(out=wq, in_=wg_d.rearrange("(g p) d -> p g d", p=P))
        pairs = []
        for i in range(GT):
            xt, wt = xq[:, i, :], wq[:, i, :]
            sumsq(xt, rgg[:, i:i + 1])
            sumsq(wt, rgg[:, GT + i:GT + i + 1])
            pairs.append((xt, wt))
        for i, (xt, wt) in enumerate(pairs):
            sq2 = scr.tile([P, D], F32, tag="sq2")
            nc.vector.scalar_tensor_tensor(
                out=sq2, in0=xt, scalar=1.0, in1=wt,
                op0=OP.mult, op1=OP.mult, accum_out=dots[:, i:i + 1])
        nc.vector.tensor_scalar_max(rgg, rgg, EPS * EPS)
        rsqrt_newton(rgg, 2 * GT)
        nc.vector.tensor_tensor(cosy, dots, rgg[:, 0:GT], OP.mult)
        nc.vector.tensor_tensor(cosy, cosy, rgg[:, GT:2 * GT], OP.mult)

    def phi_chain():
        # mphi = sin*sin_m - cosy*cos_m = -phi
        sq = sb.tile([P, GT], F32)
        nc.vector.tensor_tensor(sq, cosy, cosy, OP.mult)
        om = sb.tile([P, GT], F32)
        nc.vector.tensor_scalar(om, sq, -1.0, 1.0, OP.mult, OP.add)
        nc.vector.tensor_scalar_max(om, om, 0.0)
        rsin = sb.tile([P, GT], F32)
        nc.vector.tensor_scalar_max(rsin, om, 1e-30)
        rsqrt_newton(rsin, GT)
        sin = sb.tile([P, GT], F32)
        nc.vector.tensor_tensor(sin, om, rsin, OP.mult)
        cm = sb.tile([P, GT], F32)
        nc.vector.tensor_scalar_mul(cm, cosy, cos_m)
        mphi = sb.tile([P, GT], F32)
        nc.vector.scalar_tensor_tensor(
            out=mphi, in0=sin, scalar=sin_m, in1=cm,
            op0=OP.mult, op1=OP.subtract)

        expphi = sb.tile([P, GT], F32)
        nc.scalar.activation(out=expphi, in_=mphi, func=AF.Exp,
                             bias=nbias, scale=-S_SCALE)
        expcos = sb.tile([P, GT], F32)
        nc.scalar.activation(out=expcos, in_=cosy, func=AF.Exp,
                             bias=nbias, scale=S_SCALE)
        nc.vector.tensor_tensor(delta, expphi, expcos, OP.subtract)
        nc.vector.tensor_tensor(delta, delta, own_t, OP.mult)
        nc.vector.tensor_scalar_mul(tvec, mphi, -S_SCALE)
        nc.vector.tensor_tensor(tvec, tvec, own_t, OP.mult)

    # ---- emission schedule: PE-dense, DVE feeds one W group ahead ----
    x_group1()
    w_group(8, 14)
    w_group(14, 20)
    mm_chunk(0)
    w_group(20, 26)
    w_group(26, 32)
    mm_chunk(1)
    wg_chain()
    phi_chain()
    mm_chunk(2, last=True)

    # ---- tail: write p-major outputs ----
    nc.sync.dma_start(out=s_d.rearrange("(p m) -> p m", p=P), in_=Sacc)
    nc.sync.dma_start(out=d_d.rearrange("(p m) -> p m", p=P), in_=delta)
    nc.sync.dma_start(out=t_d.rearrange("(p m) -> p m", p=P), in_=tvec)


def build_bass():
    nc = bacc.Bacc("TRN2", target_bir_lowering=False, debug=False,
                   enable_asserts=False, num_devices=NCORES)
    x_d = nc.dram_tensor("x_in", [B, D], BF16, kind="ExternalInput").ap()
    w_d = nc.dram_tensor("w_shard", [VSP, D], BF16, kind="ExternalInput").ap()
    xg_d = nc.dram_tensor("x_gather", [GCAP, D], F32, kind="ExternalInput").ap()
    wg_d = nc.dram_tensor("w_gather", [GCAP, D], F32, kind="ExternalInput").ap()
    own_d = nc.dram_tensor("owned", [GCAP], F32, kind="ExternalInput").ap()
    s_d = nc.dram_tensor("s_out", [B], F32, kind="ExternalOutput").ap()
    d_d = nc.dram_tensor("d_out", [GCAP], F32, kind="ExternalOutput").ap()
    t_d = nc.dram_tensor("t_out", [GCAP], F32, kind="ExternalOutput").ap()
    with tile.TileContext(nc) as tc:
        _arc_kernel(tc, x_d, w_d, xg_d, wg_d, own_d, s_d, d_d, t_d)
    nc.compile()
    return nc


_NC = None


def _get_nc():
    global _NC
    if _NC is None:
        _NC = build_bass()
    return _NC


def _pm(vec, nt):
    """host-side inverse of the device's p-major [(p, m)] output layout."""
    return vec.reshape(P, nt).T.reshape(-1)


def make_in_maps(x: np.ndarray, W: np.ndarray, labels: np.ndarray):
    import ml_dtypes
    x = np.ascontiguousarray(x, dtype=np.float32)
    W = np.ascontiguousarray(W, dtype=np.float32)
    x16 = x.astype(ml_dtypes.bfloat16)
    W16 = W.astype(ml_dtypes.bfloat16)
    lab = np.asarray(labels).astype(np.int64)
    shard_of = lab // VS
    in_maps = []
    idxs = []
    for c in range(NCORES):
        wsh = np.zeros((VSP, D), dtype=ml_dtypes.bfloat16)
        wsh[:VS] = W16[c * VS:(c + 1) * VS]
        idx = np.nonzero(shard_of == c)[0]
        assert len(idx) <= GCAP, f"core {c} owns {len(idx)} rows > {GCAP}"
        idxs.append(idx)
        xg = np.zeros((GCAP, D), dtype=np.float32)
        wg = np.zeros((GCAP, D), dtype=np.float32)
        xg[:len(idx)] = x[idx]
        wg[:len(idx)] = W[lab[idx]]
        owned = np.zeros(GCAP, dtype=np.float32)
        owned[:len(idx)] = 1.0
        # device reads owned as [(p, m)] p-major
        owned_pm = owned.reshape(GCAP // P, P).T.reshape(-1).copy()
        in_maps.append({
            "x_in": x16,
            "w_shard": wsh,
            "x_gather": xg,
            "w_gather": wg,
            "owned": owned_pm,
        })
    return in_maps, idxs


def combine_outputs(results, idxs):
    S = np.zeros(B, dtype=np.float64)
    t = np.zeros(B, dtype=np.float64)
    for c, r in enumerate(results):
        S += _pm(r["s_out"], MT).astype(np.float64)
    for c, r in enumerate(results):
        idx = idxs[c]
        S[idx] += _pm(r["d_out"], GCAP // P).astype(np.float64)[:len(idx)]
        t[idx] = _pm(r["t_out"], GCAP // P).astype(np.float64)[:len(idx)]
    loss = np.mean(SHIFT + np.log(S) - t)
    return np.asarray(loss, dtype=np.float32)


def kernel(x, W, labels, **run_kwargs):
    x = np.asarray(x)
    W = np.asarray(W)
    labels = np.asarray(labels)
    assert x.shape == (B, D) and W.shape == (V, D) and labels.shape == (B,), \
        (x.shape, W.shape, labels.shape)
    nc = _get_nc()
    in_maps, idxs = make_in_maps(x, W, labels)
    res = bass_utils.run_bass_kernel_spmd(
        nc, in_maps, core_ids=list(range(NCORES)), **run_kwargs)
    out = combine_outputs(res.results, idxs)
    kernel.last_results = res
    return out
